# revision 3
# baseline (speedup 1.0000x reference)
"""BiLSTM Trainium2 kernel (Bass/Tile), SPMD over 8 NeuronCores.

Problem: x (64, 512, 512); per direction W (512, 2048), U (512, 2048), b (2048,).
Outputs: hidden_seq (64, 512, 1024), h_t (64, 1024), c_t (64, 1024).

Sharding: direction (2) x batch quarters (4) -> 8 cores, 16 batch rows each.
Each core runs ONE direction's full 512-step recurrence for its 16 rows; the
x @ W + b term is precomputed on-device as an M=128-tiled matmul (8 timesteps
per tile) and streamed back during the recurrence. No collectives.

The same Bass program runs on all 8 cores; per-core data differs (backward
cores get the time-reversed sequence and the backward weights).
"""

import numpy as np

B, S, F, H = 64, 512, 512, 512
NCORES = 8
BL = B // 4            # 16 batch rows per core
G = 4 * H              # 2048 gate columns
P = 128
KT = F // P            # 4 contraction tiles
NCH = G // 512         # 4 gate-column chunks of 512

_NC_CACHE = {}


def _build_nc(s_steps=S):
    import concourse.tile as tile
    from concourse import bacc, mybir
    from concourse.masks import make_identity

    f32 = mybir.dt.float32
    f32r = mybir.dt.float32r
    SIG = mybir.ActivationFunctionType.Sigmoid
    TANH = mybir.ActivationFunctionType.Tanh

    rows = s_steps * BL
    MT = rows // P       # precompute M-tiles (8 timesteps each)

    nc = bacc.Bacc("TRN2", debug=False, enable_asserts=False)
    xT = nc.dram_tensor("xT", (F, rows), f32r, kind="ExternalInput").ap()
    Wt = nc.dram_tensor("W", (F, G), f32r, kind="ExternalInput").ap()
    Ut = nc.dram_tensor("U", (H, G), f32r, kind="ExternalInput").ap()
    brep = nc.dram_tensor("brep", (P, G), f32, kind="ExternalInput").ap()
    hs = nc.dram_tensor("hs", (rows, H), f32, kind="ExternalOutput").ap()
    ht = nc.dram_tensor("ht", (BL, H), f32, kind="ExternalOutput").ap()
    ct = nc.dram_tensor("ct", (BL, H), f32, kind="ExternalOutput").ap()

    with tile.TileContext(nc) as tc:
        with tc.tile_pool(name="dram", bufs=1, space="DRAM") as dram_pool:
            xwb_dram = dram_pool.tile([rows, G], f32, name="xwb")

            # ---------------- phase 1: xwb = x @ W + b ----------------
            with tc.tile_pool(name="wsb", bufs=1) as wpool, \
                 tc.tile_pool(name="xtp", bufs=3) as xpool, \
                 tc.tile_pool(name="pp", bufs=8, space="PSUM") as ppool, \
                 tc.tile_pool(name="ob", bufs=4) as opool:
                W_sb = wpool.tile([P, KT, G], f32r, name="W_sb")
                nc.sync.dma_start(W_sb, Wt.rearrange("(kt p) g -> p kt g", p=P))
                brep_sb = wpool.tile([P, G], f32, name="brep_sb")
                nc.sync.dma_start(brep_sb, brep)
                xT3 = xT.rearrange("(kt p) r -> p kt r", p=P)
                for m in range(MT):
                    xt = xpool.tile([P, KT, P], f32r, name="xt", tag="xt")
                    nc.sync.dma_start(xt, xT3[:, :, m * P:(m + 1) * P])
                    for n in range(NCH):
                        pp = ppool.tile([P, 512], f32, name="pp", tag="pp")
                        for k in range(KT):
                            nc.tensor.matmul(
                                pp,
                                xt[:, k, :],
                                W_sb[:, k, n * 512:(n + 1) * 512],
                                start=(k == 0),
                                stop=(k == KT - 1),
                            )
                        ob = opool.tile([P, 512], f32, name="ob", tag="ob")
                        nc.vector.tensor_add(ob, pp, brep_sb[:, n * 512:(n + 1) * 512])
                        nc.sync.dma_start(
                            xwb_dram[m * P:(m + 1) * P, n * 512:(n + 1) * 512], ob
                        )

            # ---------------- phase 2: the recurrence ----------------
            with tc.tile_pool(name="usb", bufs=1) as upool, \
                 tc.tile_pool(name="state", bufs=1) as spool, \
                 tc.tile_pool(name="xw", bufs=4) as xwpool, \
                 tc.tile_pool(name="pg", bufs=6, space="PSUM") as pgpool, \
                 tc.tile_pool(name="pt", bufs=2, space="PSUM") as ptpool, \
                 tc.tile_pool(name="ep", bufs=2) as epool:
                U_sb = upool.tile([P, KT, G], f32r, name="U_sb")
                nc.sync.dma_start(U_sb, Ut.rearrange("(kt p) g -> p kt g", p=P))
                ident = upool.tile([BL, BL], f32, name="ident")
                make_identity(nc, ident)

                hT = []
                for k in range(KT):
                    t_ = spool.tile([P, BL], f32r, name=f"hT{k}")
                    nc.vector.memset(t_.bitcast(f32), 0.0)
                    hT.append(t_)
                c_sb = spool.tile([BL, H], f32, name="c_sb")
                nc.vector.memset(c_sb, 0.0)

                for t in range(s_steps):
                    xw = xwpool.tile([BL, G], f32, name="xw", tag="xw")
                    nc.sync.dma_start(xw, xwb_dram[t * BL:(t + 1) * BL, :])
                    gates = []
                    for n in range(NCH):
                        pg = pgpool.tile([BL, 512], f32, name="pg", tag="pg")
                        for k in range(KT):
                            nc.tensor.matmul(
                                pg,
                                hT[k],
                                U_sb[:, k, n * 512:(n + 1) * 512],
                                start=(k == 0),
                                stop=(k == KT - 1),
                            )
                        pre = epool.tile([BL, 512], f32, name="pre", tag=f"pre{n}")
                        nc.vector.tensor_add(pre, pg, xw[:, n * 512:(n + 1) * 512])
                        gt = epool.tile([BL, 512], f32, name="gt", tag=f"gt{n}")
                        nc.scalar.activation(gt, pre, TANH if n == 2 else SIG)
                        gates.append(gt)
                    i_g, f_g, g_g, o_g = gates
                    fc = epool.tile([BL, H], f32, name="fc", tag="fc")
                    nc.vector.tensor_mul(fc, f_g, c_sb)
                    ig = epool.tile([BL, H], f32, name="ig", tag="ig")
                    nc.vector.tensor_mul(ig, i_g, g_g)
                    nc.vector.tensor_add(c_sb, fc, ig)
                    tanc = epool.tile([BL, H], f32, name="tanc", tag="tanc")
                    nc.scalar.activation(tanc, c_sb, TANH)
                    h_sb = epool.tile([BL, H], f32, name="h_sb", tag="h_sb")
                    nc.vector.tensor_mul(h_sb, o_g, tanc)
                    nc.sync.dma_start(hs[t * BL:(t + 1) * BL, :], h_sb)
                    for k in range(KT):
                        pt = ptpool.tile([P, BL], f32, name="pt", tag="pt")
                        nc.tensor.transpose(pt, h_sb[:, k * P:(k + 1) * P], ident)
                        nc.vector.tensor_copy(hT[k], pt)
                    if t == s_steps - 1:
                        nc.sync.dma_start(ht, h_sb)
                        nc.sync.dma_start(ct, c_sb)

    nc.compile()
    return nc


def get_nc(s_steps=S):
    if s_steps not in _NC_CACHE:
        _NC_CACHE[s_steps] = _build_nc(s_steps)
    return _NC_CACHE[s_steps]


def make_in_maps(x, Wf, Uf, bf, Wb, Ub, bb, s_steps=S):
    """Per-core input dicts. Cores 0-3: forward, batch quarters; 4-7: backward."""
    x = np.ascontiguousarray(np.asarray(x, dtype=np.float32))
    params = {
        0: (np.asarray(Wf, np.float32), np.asarray(Uf, np.float32),
            np.asarray(bf, np.float32)),
        1: (np.asarray(Wb, np.float32), np.asarray(Ub, np.float32),
            np.asarray(bb, np.float32)),
    }
    in_maps = []
    for core in range(NCORES):
        d, q = core // 4, core % 4
        xs = x[BL * q:BL * (q + 1)]           # (BL, S, F)
        if d == 1:
            xs = xs[:, ::-1, :]
        xs = xs[:, :s_steps, :]
        xTc = np.ascontiguousarray(
            xs.transpose(2, 1, 0).reshape(F, s_steps * BL))
        W, U, b = params[d]
        in_maps.append({
            "xT": xTc,
            "W": np.ascontiguousarray(W),
            "U": np.ascontiguousarray(U),
            "brep": np.ascontiguousarray(np.broadcast_to(b, (P, G))),
        })
    return in_maps


def assemble(results, s_steps=S):
    """Gather per-core outputs into full (hidden_seq, h_t, c_t)."""
    hidden_seq = np.empty((B, s_steps, 2 * H), np.float32)
    htf = np.empty((B, H), np.float32)
    htb = np.empty((B, H), np.float32)
    ctf = np.empty((B, H), np.float32)
    for core in range(NCORES):
        d, q = core // 4, core % 4
        r = results[core]
        hsv = r["hs"].reshape(s_steps, BL, H).transpose(1, 0, 2)
        sl = slice(BL * q, BL * (q + 1))
        if d == 0:
            hidden_seq[sl, :, :H] = hsv
            htf[sl] = r["ht"]
            ctf[sl] = r["ct"]
        else:
            hidden_seq[sl, :, H:] = hsv
            htb[sl] = r["ht"]
    h_t = np.concatenate([htf, htb], axis=1)
    c_t = np.concatenate([ctf, ctf], axis=1)
    return hidden_seq, h_t, c_t


def kernel(x, Wf, Uf, bf, Wb, Ub, bb):
    from concourse.bass_utils import run_bass_kernel_spmd

    nc = get_nc(S)
    in_maps = make_in_maps(x, Wf, Uf, bf, Wb, Ub, bb, S)
    res = run_bass_kernel_spmd(nc, in_maps, core_ids=list(range(NCORES)))
    return assemble(res.results, S)
